# revision 14
# baseline (speedup 1.0000x reference)
"""Multi-Head Latent Attention forward on 8 trn2 NeuronCores (Bass/Tile).

Sharding: core c handles (batch b = c//2, head-half hh = c%2).  Each core
computes, for its batch's full sequence, the shared latent downsample, the
K/V upsample + Q projection for its 8 heads, full softmax attention, and a
partial (8-head) output projection.  Host sums the two head-half partials
per batch.  b_o is added on device by the hh==0 cores (the hh==1 cores
receive a zero bias).

On-chip layout is "transposed activation" land: activations are stored
[feature, seq] so every matmul contraction lands on the partition dim and
no transposes are needed anywhere:

  xT[dm,s] -> latentsT[l,s] -> keysT[d,s] (per head)          (PE)
           -> qT[hd,s] (spilled to DRAM, reloaded per head)   (PE)
  scoresT[k,q] = keysT.T @ qT        exp on ACT (scale=1/sqrt(dh))
  ctxT[d,q]   = values.T @ exp       values built [s,d] natural
  sums[1,q]   = ones.T @ exp         (PSUM-accumulated, M=1 matmuls)
  out[q,dm]   = ctxT.T @ W_o + b_o   (natural layout, contiguous DMA out)

All matmul operands are float32r (full PE rate at N>=256, ~tf32 rounding);
PSUM accumulation is fp32.  Weights are pre-arranged on the host into the
exact SBUF layout ([128, free]) so weight DMAs are 128 large contiguous
descriptors.  Phases: P0 latents+q (one pass over xT, qT spilled), P1 per
4-head group K/V build + attention (ctxT spilled; latentsT freed before the
last group's attention to make room for the W_o prefetch), P2 output
projection streaming ctxT back per 128-query stripe.
"""

import contextlib

import numpy as np

B, S, DM, DL, H, DH = 4, 2048, 2048, 512, 16, 128
HH = 8           # heads per core
N_CORES = 8
QT = 256         # query tile (attention)
ST = 256         # seq stripe (projections)
INV_SQRT_DH = 1.0 / np.sqrt(np.float32(DH))

_cache = {}


def _build():
    import concourse.bass as bass
    import concourse.mybir as mybir
    import concourse.tile as tile
    from concourse import bacc

    dt = mybir.dt
    f32, f32r = dt.float32, dt.float32r
    AF = mybir.ActivationFunctionType

    n_st = S // ST          # 8 stripes
    n_qt = S // QT          # 8 q tiles per head
    n_kc = S // 128         # 16 key chunks
    n_dmc = DM // 128       # 16 model-dim chunks
    n_lc = DL // 128        # 4 latent chunks

    nc = bacc.Bacc("TRN2", target_bir_lowering=False, debug=False,
                   num_devices=N_CORES)

    # inputs; weights/biases come pre-arranged in SBUF layout [128, free]
    xT = nc.dram_tensor("xT", [DM, S], f32r, kind="ExternalInput")
    w_down = nc.dram_tensor("w_down", [128, n_lc, n_dmc * 128], f32r, kind="ExternalInput")
    w_q = nc.dram_tensor("w_q", [128, 2, n_dmc * 512], f32r, kind="ExternalInput")
    w_uk = nc.dram_tensor("w_uk", [128, n_lc * HH * DH], f32r, kind="ExternalInput")
    w_uv = nc.dram_tensor("w_uv", [128, n_lc * HH * DH], f32r, kind="ExternalInput")
    w_o = nc.dram_tensor("w_o", [128, 2, HH * 1024], f32r, kind="ExternalInput")
    b_down = nc.dram_tensor("b_down", [128, n_lc], f32, kind="ExternalInput")
    b_q = nc.dram_tensor("b_q", [128, HH], f32, kind="ExternalInput")
    b_uk = nc.dram_tensor("b_uk", [128, HH], f32, kind="ExternalInput")
    b_uv = nc.dram_tensor("b_uv", [128, HH], f32, kind="ExternalInput")
    b_o = nc.dram_tensor("b_o", [DM], f32r, kind="ExternalInput")
    out = nc.dram_tensor("out", [S, DM], f32, kind="ExternalOutput")

    qT_s = nc.dram_tensor("qT_s", [HH * DH, S], f32r)      # scratch
    ctxT_s = nc.dram_tensor("ctxT_s", [HH * DH, S], f32r)  # scratch

    xT_r = xT.rearrange("(t p) s -> p t s", p=128)
    qTs_r = qT_s.rearrange("(t p) s -> p t s", p=128)
    ctxs_r = ctxT_s.rearrange("(t p) s -> p t s", p=128)

    with tile.TileContext(nc, pool_alloc_mode="queue") as tc:
        with contextlib.ExitStack() as ctx:
            pp = ctx.enter_context(tc.tile_pool(name="persist", bufs=1))
            onescol = pp.tile([128, 1], f32r, tag="ones_c")
            onesrow = pp.tile([1, 512], f32r, tag="ones_r")
            ones_f = pp.tile([128, 1], f32, tag="ones_f")
            ones_f2 = pp.tile([1, 512], f32, tag="ones_f2")
            buv_sb = pp.tile([128, HH], f32, tag="buv")
            nc.gpsimd.memset(ones_f[:], 1.0)
            nc.gpsimd.memset(ones_f2[:], 1.0)
            nc.vector.tensor_copy(onescol[:], ones_f[:])
            nc.vector.tensor_copy(onesrow[:], ones_f2[:])
            nc.sync.dma_start(buv_sb[:], b_uv[:, :])

            latp = tc.alloc_tile_pool(name="latp", bufs=1)
            latentsT = latp.tile([128, n_lc, S], f32r, tag="latT")
            wuk_sb = latp.tile([128, n_lc, HH * DH], f32r, tag="wuk")
            buk_sb = latp.tile([128, HH], f32, tag="buk")

            # ---------------- P0: latentsT + qT (one pass over xT) --------
            with tc.tile_pool(name="pq", bufs=1) as pqp:
                with tc.tile_pool(name="p0w", bufs=1) as wp, \
                     tc.tile_pool(name="p0x", bufs=2) as xp, \
                     tc.tile_pool(name="p0s", bufs=1) as sp, \
                     tc.tile_pool(name="p0ps", bufs=3, space="PSUM") as pps:
                    # DMA emission order == sync queue order: wdown (split
                    # by l-tile), first x stripe, then wq halves, wuk.
                    wdown_sb = wp.tile([128, n_lc, n_dmc * 128], f32r, tag="wdown")
                    bdown_sb = wp.tile([128, n_lc], f32, tag="bdown")
                    for lt in range(n_lc):
                        nc.sync.dma_start(wdown_sb[:, lt, :], w_down[:, lt, :])
                    nc.sync.dma_start(bdown_sb[:], b_down[:, :])

                    xts = {}

                    def get_xt(st):
                        if st not in xts:
                            t = xp.tile([128, n_dmc, ST], f32r, tag="xt",
                                        name=f"xt{st}")
                            nc.sync.dma_start(t[:], xT_r[:, :, bass.ts(st, ST)])
                            xts[st] = t
                        return xts[st]

                    get_xt(0)
                    wq_sb = pqp.tile([128, 2, n_dmc * 512], f32r, tag="wq")
                    bq_sb = pqp.tile([128, HH], f32, tag="bq")
                    nc.sync.dma_start(wq_sb[:, 0, :], w_q[:, 0, :])
                    nc.sync.dma_start(wq_sb[:, 1, :], w_q[:, 1, :])
                    nc.sync.dma_start(bq_sb[:], b_q[:, :])
                    nc.sync.dma_start(wuk_sb[:], w_uk[:, :])
                    nc.sync.dma_start(buk_sb[:], b_uk[:, :])

                    for st in range(n_st):
                        ssl = bass.ts(st, ST)
                        xt = get_xt(st)
                        for lt in range(n_lc):
                            ps = pps.tile([128, ST], f32, tag="lat")
                            for c in range(n_dmc):
                                nc.tensor.matmul(ps[:], wdown_sb[:, lt, bass.ts(c, 128)],
                                                 xt[:, c, :], start=(c == 0), stop=(c == n_dmc - 1))
                            nc.scalar.activation(latentsT[:, lt, ssl], ps[:], AF.Identity,
                                                 bias=bdown_sb[:, lt:lt + 1])
                        if st + 1 < n_st:
                            get_xt(st + 1)
                        qstage = sp.tile([128, HH, ST], f32r, tag="qs")
                        for ht in range(HH):
                            ps = pps.tile([128, ST], f32, tag="q")
                            for c in range(n_dmc):
                                nc.tensor.matmul(
                                    ps[:], wq_sb[:, ht // 4, bass.ds((ht % 4) * n_dmc * 128 + c * 128, 128)],
                                    xt[:, c, :], start=(c == 0), stop=(c == n_dmc - 1))
                            nc.scalar.activation(qstage[:, ht, :], ps[:], AF.Identity,
                                                 bias=bq_sb[:, ht:ht + 1])
                        nc.sync.dma_start(qTs_r[:, :, ssl], qstage[:])

            # ------------ P1: per 4-head group: K/V build + attention -----
            ctx2 = ctx.enter_context(contextlib.ExitStack())
            kvp = ctx2.enter_context(tc.tile_pool(name="p1kv", bufs=1))
            keysT = kvp.tile([128, 4, S], f32r, tag="keysT")
            values = kvp.tile([128, n_kc, 512], f32r, tag="values")
            wuv_sb = kvp.tile([128, n_lc, HH * DH], f32r, tag="wuv")
            nc.sync.dma_start(wuv_sb[:], w_uv[:, :])

            def kv_build(g, kvps):
                for dt_ in range(4):
                    for st in range(n_st):
                        ps = kvps.tile([128, ST], f32, tag="kv", name=f"kps{g}{dt_}{st}")
                        for lt in range(n_lc):
                            nc.tensor.matmul(
                                ps[:], wuk_sb[:, lt, bass.ds(g * 512 + dt_ * 128, 128)],
                                latentsT[:, lt, bass.ts(st, ST)],
                                start=(lt == 0), stop=(lt == n_lc - 1))
                        nc.scalar.activation(keysT[:, dt_, bass.ts(st, ST)], ps[:],
                                             AF.Identity,
                                             bias=buk_sb[:, g * 4 + dt_:g * 4 + dt_ + 1])
                for sc in range(n_kc):
                    ps = kvps.tile([128, 512], f32, tag="kv", name=f"vps{g}{sc}")
                    for lt in range(n_lc):
                        nc.tensor.matmul(ps[:], latentsT[:, lt, bass.ts(sc, 128)],
                                         wuv_sb[:, lt, bass.ds(g * 512, 512)],
                                         start=(lt == 0), stop=(lt == n_lc - 1))
                    nc.scalar.activation(values[:, sc, :], ps[:], AF.Identity)

            def attention(g, qp, ep, sp2, scp, avps, bcps):
                for hl in range(4):
                    h = g * 4 + hl
                    qh = qp.tile([128, S], f32r, tag="qh", name=f"qh{h}")
                    nc.sync.dma_start(qh[:], qT_s[bass.ts(h, 128), :])
                    for qt in range(n_qt):
                        qsl = bass.ts(qt, QT)
                        eb = ep.tile([128, n_kc, QT], f32r, tag="eb", name=f"eb{h}{qt}")
                        for blk in range(4):
                            ps = scp.tile([128, 4, QT], f32, tag="sc", name=f"sc{h}{qt}{blk}")
                            for c4 in range(4):
                                c = blk * 4 + c4
                                nc.tensor.matmul(ps[:, c4, :],
                                                 keysT[:, hl, bass.ts(c, 128)],
                                                 qh[:, qsl], start=True, stop=True)
                            nc.scalar.activation(eb[:, blk * 4:(blk + 1) * 4, :],
                                                 ps[:], AF.Exp, scale=INV_SQRT_DH)
                        avp = avps.tile([128, QT], f32, tag="av", name=f"av{h}{qt}")
                        for c in range(n_kc):
                            nc.tensor.matmul(avp[:], values[:, c, bass.ts(hl, 128)],
                                             eb[:, c, :], start=(c == 0),
                                             stop=(c == n_kc - 1))
                        bcp = bcps.tile([128, QT], f32, tag="bc", name=f"bc{h}{qt}")
                        for c in range(n_kc):
                            nc.tensor.matmul(bcp[0:1, :], onescol[:], eb[:, c, :],
                                             start=(c == 0), stop=(c == n_kc - 1))
                        recip = sp2.tile([1, QT], f32, tag="rc")
                        nc.vector.reciprocal_approx_fast(recip[:], bcp[0:1, :])
                        recipr = sp2.tile([1, QT], f32r, tag="rcr")
                        nc.vector.tensor_scalar_add(recipr[:], recip[:], 0.0)
                        nc.tensor.matmul(bcp[:], onesrow[:, :128], recipr[:],
                                         start=True, stop=True)
                        bcs = sp2.tile([128, QT], f32, tag="bcs")
                        nc.vector.tensor_copy(bcs[:], bcp[:])
                        tmp = sp2.tile([128, QT], f32, tag="tmp")
                        nc.vector.tensor_mul(tmp[:], avp[:], bcs[:])
                        ctxs = sp2.tile([128, QT], f32r, tag="ctxs")
                        nc.vector.tensor_scalar_add(ctxs[:], tmp[:], buv_sb[:, h:h + 1])
                        nc.sync.dma_start(ctxT_s[bass.ts(h, 128), qsl], ctxs[:])

            with tc.tile_pool(name="p1q", bufs=2) as qp, \
                 tc.tile_pool(name="p1e", bufs=2) as ep, \
                 tc.tile_pool(name="p1s", bufs=3) as sp2:
                for g in range(2):
                    with tc.tile_pool(name=f"kvps{g}", bufs=2, space="PSUM") as kvps:
                        kv_build(g, kvps)
                    with tc.tile_pool(name=f"scp{g}", bufs=3, space="PSUM") as scp, \
                         tc.tile_pool(name=f"avp{g}", bufs=1, space="PSUM") as avps, \
                         tc.tile_pool(name=f"bcp{g}", bufs=1, space="PSUM") as bcps:
                        attention(g, qp, ep, sp2, scp, avps, bcps)
            ctx2.close()
            latp.release()

            # ---------------- P2: output projection (natural layout) ------
            # ctx resident (quarter-loaded); W_o halves so phase A starts
            # after only half the weights have landed.
            with tc.tile_pool(name="p2w", bufs=1) as wop, \
                 tc.tile_pool(name="p2c", bufs=1) as cp, \
                 tc.tile_pool(name="p2s", bufs=3) as osp, \
                 tc.tile_pool(name="p2ps", bufs=2, space="PSUM") as ops:
                bo_sb = wop.tile([1, DM], f32r, tag="bo")
                nc.sync.dma_start(bo_sb[:], b_o[None, :])
                wo_sb = wop.tile([128, 2, HH * 1024], f32r, tag="wo")
                nc.sync.dma_start(wo_sb[:, 0, :], w_o[:, 0, :])
                ctxall = cp.tile([128, HH, S], f32r, tag="ctxall")
                for qq in range(4):
                    nc.sync.dma_start(ctxall[:, :, bass.ts(qq, 512)],
                                      ctxs_r[:, :, bass.ts(qq, 512)])
                nc.sync.dma_start(wo_sb[:, 1, :], w_o[:, 1, :])
                for ph in range(2):
                    for qt in range(S // 128):
                        qsl = bass.ts(qt, 128)
                        pst = [ops.tile([128, 512], f32, tag=f"o{d2}", name=f"po{ph}{qt}{d2}")
                               for d2 in range(2)]
                        for hc in range(HH):
                            for d2 in range(2):
                                nc.tensor.matmul(
                                    pst[d2][:], ctxall[:, hc, qsl],
                                    wo_sb[:, ph, bass.ds(hc * 1024 + d2 * 512, 512)],
                                    start=(hc == 0), stop=False)
                        for d2 in range(2):
                            dmt = 2 * ph + d2
                            dsl = bass.ts(dmt, 512)
                            nc.tensor.matmul(pst[d2][:], onesrow[:, :128], bo_sb[:, dsl],
                                             start=False, stop=True)
                            ostage = osp.tile([128, 512], f32, tag="os", name=f"os{ph}{qt}{d2}")
                            nc.vector.tensor_copy(ostage[:], pst[d2][:])
                            nc.sync.dma_start(out[qsl, dsl], ostage[:])

    nc.compile()
    return nc


def _get_nc():
    if "nc" not in _cache:
        _cache["nc"] = _build()
    return _cache["nc"]


def _prep_w(w):
    """[K, F] -> SBUF layout [128, (K//128) * F], chunk-major along free."""
    k, f = w.shape
    return np.ascontiguousarray(
        w.reshape(k // 128, 128, f).transpose(1, 0, 2).reshape(128, -1))


def _in_maps(x, W_down, b_down, W_uk, b_uk, W_uv, b_uv, W_q, b_q, W_o, b_o):
    x = np.asarray(x, dtype=np.float32)
    n_dmc = DM // 128
    n_lc = DL // 128
    zeros_bo = np.zeros_like(np.asarray(b_o))

    # shared (head-independent) preps
    # w_down: [128, n_lc, n_dmc*128]: for l-tile lt, dm-chunk c:
    #   slice [:, lt, c*128:(c+1)*128] == W_down[c*128:(c+1)*128, lt*128:(lt+1)*128]
    wd = np.asarray(W_down).reshape(n_dmc, 128, n_lc, 128)
    wd = np.ascontiguousarray(wd.transpose(1, 2, 0, 3).reshape(128, n_lc, n_dmc * 128))
    bdw = np.ascontiguousarray(np.asarray(b_down).reshape(n_lc, 128).T)
    xTs = [np.ascontiguousarray(x[b].T) for b in range(B)]

    maps = []
    for c in range(N_CORES):
        b, hh = c // 2, c % 2
        hsl = slice(hh * HH * DH, (hh + 1) * HH * DH)
        # w_q: [128, 2, n_dmc*512]: half hf (4 heads), within: head ht4,
        # dm-chunk c: [:, hf, ht4*n_dmc*128 + c*128 :+128] == W_q[c-chunk, head cols]
        wq = np.asarray(W_q)[:, hsl].reshape(n_dmc, 128, 2, 4, 128)
        wq = np.ascontiguousarray(wq.transpose(1, 2, 3, 0, 4).reshape(128, 2, n_dmc * 512))
        # w_o: [1024 hd, 2048 dm] -> [128, 2 dm-half, HH*1024]:
        #   [:, half, hc*1024 + j] == W_o[hc*128+p, half*1024 + j]
        wo = np.asarray(W_o)[hsl, :].reshape(HH, 128, 2, 1024)
        wo = np.ascontiguousarray(wo.transpose(1, 2, 0, 3).reshape(128, 2, HH * 1024))
        maps.append({
            "xT": xTs[b],
            "w_down": wd,
            "w_q": wq,
            "w_uk": _prep_w(np.asarray(W_uk)[:, hsl]),
            "w_uv": _prep_w(np.asarray(W_uv)[:, hsl]),
            "w_o": wo,
            "b_down": bdw,
            "b_q": np.ascontiguousarray(np.asarray(b_q)[hsl].reshape(HH, 128).T),
            "b_uk": np.ascontiguousarray(np.asarray(b_uk)[hsl].reshape(HH, 128).T),
            "b_uv": np.ascontiguousarray(np.asarray(b_uv)[hsl].reshape(HH, 128).T),
            "b_o": np.asarray(b_o) if hh == 0 else zeros_bo,
        })
    return maps


def kernel(x, W_down, b_down, W_uk, b_uk, W_uv, b_uv, W_q, b_q, W_o, b_o):
    from concourse.bass_utils import run_bass_kernel_spmd

    nc = _get_nc()
    maps = _in_maps(x, W_down, b_down, W_uk, b_uk, W_uv, b_uv, W_q, b_q, W_o, b_o)
    res = run_bass_kernel_spmd(nc, maps, list(range(N_CORES)))
    full = np.empty((B, S, DM), np.float32)
    for b in range(B):
        full[b] = res.results[2 * b]["out"] + res.results[2 * b + 1]["out"]
    return full
